# Initial kernel scaffold
#
"""GATv2 (3-layer, 8-head) on 8 Trainium2 NeuronCores.

Strategy (edge-parallel, dst-sharded):
- Core c owns destination nodes [c*N/8, (c+1)*N/8) and all edges into them.
- Host sorts each core's edges by (dst-window, src-half), pads to 128-edge
  chunks with a chunk structure made uniform across cores (SPMD: one program).
- Per layer: fs = h@Wsrc for ALL nodes (layer 0: replicated GEMM from the
  replicated features; layers 1/2: sharded GEMM + AllGather), fd = h@Wdst for
  the local shard only.
- Edge phase per 128-dst window: dma_gather fs[src] rows (the only per-edge
  gather), expand fd[dst] via one-hot matmul, score
  s = attn . leaky_relu(fs+fd) via DVE mul+reduce, ex = exp(s) (no segment-max:
  scores are O(1) so exp is safe), unnormalized aggregation
  rstU = OneHot @ [ex*z | ex] via TensorE (denominator rides in the last 8
  columns), then per-window normalization rst = rstU/denom - fd (using
  sum_e ex*fd[dst] = denom*fd[v]) + residual, relu.
- Output: mean over heads, host concatenates the 8 dst shards.
"""
import sys
sys.path.insert(0, "/opt/trn_rl_repo")
import numpy as np
import concourse.bass as bass
import concourse.mybir as mybir
import concourse.tile as tile
from concourse import bacc
from concourse.bass_utils import run_bass_kernel_spmd

P = 128
NCORE = 8
SLOPE = 0.2
H = 8

F32 = mybir.dt.float32
BF16 = mybir.dt.bfloat16
I16 = mybir.dt.int16
AX = mybir.AxisListType
OP = mybir.AluOpType
AF = mybir.ActivationFunctionType


# ---------------------------------------------------------------- host layout
def build_layout(src, dst, N):
    """Edge layout: per-core, dst-window-sorted, src-half-split, 128-padded,
    chunk structure uniform across cores."""
    SH = N // NCORE
    NW = (SH + P - 1) // P
    HALF = min(32768, (N + 1) // 2)  # src-half split point (int16 idx range)
    cores = []
    for c in range(NCORE):
        m = (dst // SH) == c
        s, d = src[m], dst[m]
        dl = d - c * SH
        w = dl // P
        hf = (s >= HALF).astype(np.int64)
        order = np.lexsort((hf, w))
        s, dl, w, hf = s[order], dl[order], w[order], hf[order]
        groups = {}
        for wi in range(NW):
            for h in range(2):
                gm = (w == wi) & (hf == h)
                groups[(wi, h)] = (s[gm], dl[gm])
        cores.append(groups)
    # uniform chunk counts per (window, half)
    C = {}
    for wi in range(NW):
        for h in range(2):
            n = max(len(cores[c][(wi, h)][0]) for c in range(NCORE))
            C[(wi, h)] = (n + P - 1) // P
    NCH = sum(C.values())
    # flat edge arrays per core
    src_rel = np.zeros((NCORE, NCH * P), np.int16)
    dstw = np.full((NCORE, NCH * P), -1.0, np.float32)
    calls = []  # (half, chunk_off, n_chunks) shared structure
    off = 0
    for wi in range(NW):
        for h in range(2):
            nch = C[(wi, h)]
            g = 0
            while g < nch:
                gs = min(8, nch - g)
                calls.append((wi, h, off + g, gs))
                g += gs
            for c in range(NCORE):
                s, dl = cores[c][(wi, h)]
                k = off * P
                src_rel[c, k:k + len(s)] = (s - h * HALF).astype(np.int16)
                dstw[c, k:k + len(dl)] = (dl % P).astype(np.float32)
            off += nch
    assert off == NCH
    # wrapped int16 idx for dma_gather: per call, idx i -> [i%16, col+i//16]
    TOTC = NCH * P // 16
    idx_w = np.zeros((NCORE, P, TOTC), np.int16)
    for c in range(NCORE):
        w16 = src_rel[c].reshape(-1, 16).T  # [16, NCH*8]
        idx_w[c] = np.tile(w16, (8, 1))
    # chunk-column-major dstw: [P, NCH], edge k*P+p -> [p, k]
    dstw_cols = dstw.reshape(NCORE, NCH, P).transpose(0, 2, 1).copy()
    return dict(SH=SH, NW=NW, HALF=HALF, NCH=NCH, C=C, calls=calls,
                idx_w=idx_w, dstw_cols=dstw_cols)


# ---------------------------------------------------------------- bass kernel
def build_kernel(N, IN, L):
    """L = layout dict. IN = input feature dim (128)."""
    D = 256  # H*HID = H*OUT
    SH, NW, NCH = L["SH"], L["NW"], L["NCH"]
    HALF = L["HALF"]
    T0 = (N + P - 1) // P            # full-N tiles (layer-0 fs GEMM)
    SHP = NW * P                      # padded shard rows
    TOTC = NCH * P // 16

    nc = bacc.Bacc("TRN2", target_bir_lowering=False, debug=False,
                   num_devices=NCORE)
    # ---- inputs (per-core where noted)
    featT = nc.declare_dram_parameter("featT", [IN, N], F32, isOutput=False)
    featT_loc = nc.declare_dram_parameter("featT_loc", [IN, SH], F32, isOutput=False)
    Ws = [nc.declare_dram_parameter(f"Wsrc{l}", [IN if l == 0 else D, D], F32, isOutput=False) for l in range(3)]
    Wd = [nc.declare_dram_parameter(f"Wdst{l}", [IN if l == 0 else D, D], F32, isOutput=False) for l in range(3)]
    Wres1 = nc.declare_dram_parameter("Wres1", [IN, D], F32, isOutput=False)
    attn4 = [nc.declare_dram_parameter(f"attn4_{l}", [P, 4 * D], F32, isOutput=False) for l in range(3)]
    iota4 = nc.declare_dram_parameter("iota4", [P, 4 * P], F32, isOutput=False)
    ident_in = nc.declare_dram_parameter("ident", [P, P], F32, isOutput=False)
    idx_in = nc.declare_dram_parameter("idx_w", [P, TOTC], I16, isOutput=False)   # per-core
    dstw_in = nc.declare_dram_parameter("dstw", [P, NCH], F32, isOutput=False)    # per-core
    out_ext = nc.declare_dram_parameter("out", [SH, 32], F32, isOutput=True)

    with tile.TileContext(nc) as tc:
        with (
            tc.tile_pool(name="const", bufs=1) as cpool,
            tc.tile_pool(name="sbuf", bufs=4) as sb,
            tc.tile_pool(name="sb6", bufs=4) as sb6,
            tc.tile_pool(name="sb2", bufs=3) as sb2,
            tc.tile_pool(name="psum", bufs=2, space="PSUM") as ps,
            tc.tile_pool(name="dram", bufs=1, space="DRAM") as dr,
        ):
            # ---- persistent constants
            ident = cpool.tile([P, P], F32)
            nc.sync.dma_start(out=ident[:], in_=ident_in[:, :])
            ident16 = cpool.tile([P, P], BF16, tag="ident16")
            nc.vector.tensor_copy(out=ident16[:], in_=ident[:])
            iota_t = cpool.tile([P, 4 * P], F32)
            nc.sync.dma_start(out=iota_t[:], in_=iota4[:, :])
            idx_t = cpool.tile([P, TOTC], I16)
            nc.sync.dma_start(out=idx_t[:], in_=idx_in[:, :])
            dstw_t = cpool.tile([P, NCH], F32)
            nc.sync.dma_start(out=dstw_t[:], in_=dstw_in[:, :])
            attn_t = []
            for l in range(3):
                a32 = cpool.tile([P, 4 * D], F32, tag=f"attn32_{l}")
                nc.sync.dma_start(out=a32[:], in_=attn4[l][:, :])
                a = cpool.tile([P, 4 * D], BF16, tag=f"attn{l}")
                nc.vector.tensor_copy(out=a[:], in_=a32[:])
                attn_t.append(a)
            hT = cpool.tile([P, 2, SHP], F32, tag="hT")  # local shard, transposed

            # ---- DRAM internals
            fs_full0 = dr.tile([T0 * P, D], BF16, tag="fsf0")
            fs_full1 = dr.tile([N, D], BF16, tag="fsf1", addr_space="Shared")
            fs_full2 = dr.tile([N, D], BF16, tag="fsf2", addr_space="Shared")
            fs_full_l = [None, fs_full1, fs_full2]
            ag_in = dr.tile([SH, D], BF16, tag="agin")
            fd_dram = dr.tile([SHP, D], BF16, tag="fd")
            res_dram = dr.tile([SHP, D], F32, tag="res")
            h_dram = dr.tile([SHP, D], F32, tag="h")

            zero_sb = cpool.tile([P, D], F32, tag="zero")
            nc.vector.memset(zero_sb[:], 0.0)
            zero16 = cpool.tile([P, D], BF16, tag="zero16")
            nc.vector.memset(zero16[:], 0.0)
            if SHP > SH:  # zero the padded tails once
                nc.sync.dma_start(out=fd_dram[SH:SHP, :], in_=zero16[:SHP - SH, :])
                nc.sync.dma_start(out=res_dram[SH:SHP, :], in_=zero_sb[:SHP - SH, :])
                nc.sync.dma_start(out=h_dram[SH:SHP, :], in_=zero_sb[:SHP - SH, :])

            def gemm(lhsT_ap_fn, kchunks, rhs_t, mt, out_psum):
                """out_psum[mt, D] = sum_k lhsT_k.T @ rhs_k"""
                for k in range(kchunks):
                    nc.tensor.matmul(out_psum[:mt, :D], lhsT=lhsT_ap_fn(k),
                                     rhs=rhs_t[:, k, :],
                                     start=(k == 0), stop=(k == kchunks - 1))

            def load_w(wparam, kchunks, tag):
                wt = cpool.tile([P, kchunks, D], F32, tag=tag)
                nc.sync.dma_start(
                    out=wt[:], in_=wparam.ap().rearrange("(c k) n -> k c n", k=P))
                return wt

            wsrc_t = [load_w(Ws[l], 1 if l == 0 else 2, f"wsrc{l}") for l in range(3)]
            wdst_t = [load_w(Wd[l], 1 if l == 0 else 2, f"wdst{l}") for l in range(3)]
            wres_t = load_w(Wres1, 1, "wres")

            for l in range(3):
                DIN = IN if l == 0 else D
                KCH = DIN // P
                act_relu = l < 2
                # ================= dense phase =================
                if l == 0:
                    # replicated fs_full GEMM from featT
                    for t in range(T0):
                        mt = min(P, N - t * P)
                        lt = sb.tile([P, P], F32, tag="lhsT")
                        nc.sync.dma_start(out=lt[:, :mt], in_=featT[:, t * P:t * P + mt])
                        pst = ps.tile([P, 264], F32, tag="rstcat", space="PSUM")
                        nc.tensor.matmul(pst[:mt, :D], lhsT=lt[:, :mt],
                                         rhs=wsrc_t[0][:, 0, :], start=True, stop=True)
                        ot = sb.tile([P, D], BF16, tag="gout")
                        nc.scalar.copy(out=ot[:mt, :], in_=pst[:mt, :D])
                        nc.sync.dma_start(out=fs_full0[t * P:t * P + mt, :], in_=ot[:mt, :])
                    # sharded fd / res GEMMs from featT_loc
                    for w in range(NW):
                        mt = min(P, SH - w * P)
                        lt = sb.tile([P, P], F32, tag="lhsT")
                        nc.sync.dma_start(out=lt[:, :mt], in_=featT_loc[:, w * P:w * P + mt])
                        for (rhs_t, dest, dt_) in ((wdst_t[0], fd_dram, BF16), (wres_t, res_dram, F32)):
                            pst = ps.tile([P, 264], F32, tag="rstcat", space="PSUM")
                            nc.tensor.matmul(pst[:mt, :D], lhsT=lt[:, :mt],
                                             rhs=rhs_t[:, 0, :], start=True, stop=True)
                            ot = sb.tile([P, D], dt_, tag="gout" if dt_ is BF16 else "gout32")
                            nc.scalar.copy(out=ot[:mt, :], in_=pst[:mt, :D])
                            nc.sync.dma_start(out=dest[w * P:w * P + mt, :], in_=ot[:mt, :])
                else:
                    # sharded fs -> ag_in; fd -> fd_dram (lhsT = resident hT)
                    for w in range(NW):
                        mt = min(P, SH - w * P)
                        for (rhs_t, dest) in ((wsrc_t[l], ag_in), (wdst_t[l], fd_dram)):
                            pst = ps.tile([P, 264], F32, tag="rstcat", space="PSUM")
                            gemm(lambda k: hT[:, k, w * P:w * P + mt], KCH, rhs_t, mt, pst)
                            ot = sb.tile([P, D], BF16, tag="gout")
                            nc.scalar.copy(out=ot[:mt, :], in_=pst[:mt, :D])
                            nc.sync.dma_start(out=dest[w * P:w * P + mt, :], in_=ot[:mt, :])
                    nc.gpsimd.collective_compute(
                        "AllGather", OP.bypass,
                        replica_groups=[list(range(NCORE))],
                        ins=[ag_in.opt()], outs=[fs_full_l[l].opt()],
                    )
                table = fs_full0 if l == 0 else fs_full_l[l]
                res_src = res_dram if l == 0 else h_dram

                # ================= edge phase =================
                cur_w = -1
                rst_ps = None
                calls = L["calls"]
                for ci, (wi, hf, koff, gcs) in enumerate(calls):
                    if wi != cur_w:
                        cur_w = wi
                        rst_ps = ps.tile([P, 264], F32, tag="rstcat", space="PSUM")
                        fdw = sb2.tile([P, D], BF16, tag="fdw")
                        nc.sync.dma_start(out=fdw[:], in_=fd_dram[wi * P:wi * P + P, :])
                        first_mm = True
                    # gather fs rows for up to 8 chunks per call
                    z8 = sb6.tile([P, 8, D], BF16, tag="z")
                    tab = table[:, :] if hf == 0 else table[HALF:, :]
                    nc.gpsimd.dma_gather(
                        z8[:, :gcs, :], tab, idx_t[:, koff * 8:koff * 8 + gcs * 8],
                        gcs * P, gcs * P, D, single_packet=False)
                    last_call = ci + 1 == len(calls) or calls[ci + 1][0] != wi
                    for sub in range(0, gcs, 4):
                        gs = min(4, gcs - sub)
                        ko = koff + sub
                        z = z8[:, sub:sub + 4, :]
                        # one-hot (edges on partitions)
                        oh = sb.tile([P, 4, P], BF16, tag="oh")
                        nc.vector.tensor_tensor(
                            out=oh[:, :gs, :],
                            in0=dstw_t[:, ko:ko + gs].to_broadcast([P, gs, P]),
                            in1=iota_t[:].rearrange("p (g j) -> p g j", g=4)[:, :gs, :],
                            op=OP.is_equal)
                        # transposed one-hot (dst on partitions) via PE
                        ohT_ps = ps.tile([P, 4 * P], BF16, tag="ohT", space="PSUM")
                        for j in range(gs):
                            nc.tensor.transpose(out=ohT_ps[:, j * P:(j + 1) * P],
                                                in_=oh[:, j, :], identity=ident16[:])
                        ohT = sb.tile([P, 4 * P], BF16, tag="ohTs")
                        nc.scalar.copy(out=ohT[:, :gs * P], in_=ohT_ps[:, :gs * P])
                        # z_psum = OneHot_ve.T @ fdw + fs  (= fs[src]+fd[dst])
                        zps = ps.tile([P, 4, D], F32, tag="zps", space="PSUM")
                        for j in range(gs):
                            nc.tensor.matmul(zps[:, j, :], lhsT=ohT[:, j * P:(j + 1) * P],
                                             rhs=fdw[:], start=True, stop=False)
                            nc.tensor.matmul(zps[:, j, :], lhsT=ident16[:],
                                             rhs=z[:, j, :], start=False, stop=True)
                        # leaky-relu = max(z, 0.2z): ACT Copy(scale) + DVE max
                        # (keeps ACT on one LUT set: Copy/Relu/Exp -> no table reloads)
                        lrs = sb.tile([P, 4, D], BF16, tag="lrs")
                        nc.scalar.activation(lrs[:, :gs, :], zps[:, :gs, :], AF.Copy,
                                             scale=SLOPE)
                        lr = sb.tile([P, 4, D], BF16, tag="lr")
                        nc.vector.tensor_tensor(out=lr[:, :gs, :], in0=zps[:, :gs, :],
                                                in1=lrs[:, :gs, :], op=OP.max)
                        sm = sb.tile([P, 4, D], BF16, tag="sm")
                        nc.vector.tensor_tensor(
                            out=sm[:, :gs, :], in0=lr[:, :gs, :],
                            in1=attn_t[l][:].rearrange("p (g d) -> p g d", g=4)[:, :gs, :],
                            op=OP.mult)
                        sc = sb.tile([P, 4, H], F32, tag="sc")
                        nc.vector.tensor_reduce(
                            out=sc[:, :gs, :],
                            in_=sm[:, :gs, :].rearrange("p g (h d) -> p g h d", h=H),
                            axis=AX.X, op=OP.add)
                        wcat = sb.tile([P, 4, 264], BF16, tag="wcat")
                        nc.scalar.activation(wcat[:, :gs, D:D + H], sc[:, :gs, :], AF.Exp)
                        # W = ex * z
                        nc.vector.tensor_tensor(
                            out=wcat[:, :gs, :D].rearrange("p g (h d) -> p g h d", h=H),
                            in0=zps[:, :gs, :].rearrange("p g (h d) -> p g h d", h=H),
                            in1=wcat[:, :gs, D:D + H].to_broadcast([P, gs, H, 32]),
                            op=OP.mult)
                        # accumulate [rstU | denom]
                        for j in range(gs):
                            last = last_call and sub + gs >= gcs and j == gs - 1
                            nc.tensor.matmul(rst_ps[:, :], lhsT=oh[:, j, :],
                                             rhs=wcat[:, j, :], start=first_mm, stop=last)
                            first_mm = False
                    # window epilogue
                    if ci + 1 == len(calls) or calls[ci + 1][0] != wi:
                        wt = min(P, SH - wi * P)
                        den = sb2.tile([P, H], F32, tag="den")
                        nc.vector.tensor_scalar_max(den[:], rst_ps[:, D:D + H], 1e-30)
                        rec = sb2.tile([P, H], F32, tag="rec")
                        nc.vector.reciprocal(rec[:], den[:])
                        msk = sb2.tile([P, H], F32, tag="msk")
                        nc.vector.tensor_scalar(out=msk[:], in0=rst_ps[:, D:D + H],
                                                scalar1=1e30, scalar2=1.0,
                                                op0=OP.mult, op1=OP.min)
                        rn = sb2.tile([P, D], F32, tag="rn")
                        nc.vector.tensor_tensor(
                            out=rn[:].rearrange("p (h d) -> p h d", h=H),
                            in0=rst_ps[:, :D].rearrange("p (h d) -> p h d", h=H),
                            in1=rec[:].to_broadcast([P, H, 32]), op=OP.mult)
                        fdw32 = sb2.tile([P, D], F32, tag="fdw32")
                        nc.vector.tensor_copy(out=fdw32[:], in_=fdw[:])
                        fdm = sb2.tile([P, D], F32, tag="fdm")
                        nc.vector.tensor_tensor(
                            out=fdm[:].rearrange("p (h d) -> p h d", h=H),
                            in0=fdw32[:].rearrange("p (h d) -> p h d", h=H),
                            in1=msk[:].to_broadcast([P, H, 32]), op=OP.mult)
                        nc.vector.tensor_tensor(out=rn[:], in0=rn[:], in1=fdm[:],
                                                op=OP.subtract)
                        rt = sb2.tile([P, D], F32, tag="rt")
                        nc.sync.dma_start(out=rt[:], in_=res_src[wi * P:wi * P + P, :])
                        nc.vector.tensor_tensor(out=rn[:], in0=rn[:], in1=rt[:], op=OP.add)
                        hsb = sb2.tile([P, D], F32, tag="hsb")
                        if act_relu:
                            nc.scalar.activation(hsb[:], rn[:], AF.Relu)
                        else:
                            nc.vector.tensor_copy(out=hsb[:], in_=rn[:])
                        if l < 2:
                            nc.sync.dma_start(out=h_dram[wi * P:wi * P + wt, :],
                                              in_=hsb[:wt, :])
                            for half in range(2):
                                tp = ps.tile([P, 4 * P], F32, tag="ohT", space="PSUM")
                                nc.tensor.transpose(out=tp[:, :P],
                                                    in_=hsb[:, half * P:(half + 1) * P],
                                                    identity=ident[:])
                                nc.scalar.copy(out=hT[:, half, wi * P:(wi + 1) * P],
                                               in_=tp[:, :P])
                        else:
                            mean = sb2.tile([P, 32], F32, tag="mean")
                            nc.vector.tensor_reduce(
                                out=mean[:],
                                in_=hsb[:].rearrange("p (h d) -> p d h", h=H),
                                axis=AX.X, op=OP.add)
                            osb = sb2.tile([P, 32], F32, tag="osb")
                            nc.scalar.mul(osb[:], mean[:], 1.0 / H)
                            nc.sync.dma_start(out=out_ext[wi * P:wi * P + wt, :],
                                              in_=osb[:wt, :])
    nc.compile()
    return nc


# ---------------------------------------------------------------- host driver
def prep_inputs(features, src, dst, Wsrc1, Wdst1, attn1, Wres1,
                Wsrc2, Wdst2, attn2, Wsrc3, Wdst3, attn3):
    N, IN = features.shape
    L = build_layout(np.asarray(src), np.asarray(dst), N)
    featT = np.ascontiguousarray(np.asarray(features).T)
    SH = L["SH"]

    def attn_rep(a):
        flat = np.asarray(a).reshape(-1)  # [256]
        return np.tile(np.tile(flat, 4)[None, :], (P, 1)).astype(np.float32)

    iota = np.tile(np.arange(P, dtype=np.float32)[None, :], (P, 4))
    ident = np.eye(P, dtype=np.float32)
    common = {
        "featT": featT, "ident": ident, "iota4": iota,
        "Wsrc0": np.asarray(Wsrc1), "Wdst0": np.asarray(Wdst1), "Wres1": np.asarray(Wres1),
        "Wsrc1": np.asarray(Wsrc2), "Wdst1": np.asarray(Wdst2),
        "Wsrc2": np.asarray(Wsrc3), "Wdst2": np.asarray(Wdst3),
        "attn4_0": attn_rep(attn1), "attn4_1": attn_rep(attn2), "attn4_2": attn_rep(attn3),
    }
    in_maps = []
    for c in range(NCORE):
        m = dict(common)
        m["featT_loc"] = np.ascontiguousarray(featT[:, c * SH:(c + 1) * SH])
        m["idx_w"] = L["idx_w"][c]
        m["dstw"] = L["dstw_cols"][c]
        in_maps.append(m)
    return L, in_maps


_BUILD_CACHE = {}


def run(features, src, dst, Wsrc1, Wdst1, attn1, Wres1,
        Wsrc2, Wdst2, attn2, Wsrc3, Wdst3, attn3, trace=False):
    N, IN = features.shape
    L, in_maps = prep_inputs(features, src, dst, Wsrc1, Wdst1, attn1, Wres1,
                             Wsrc2, Wdst2, attn2, Wsrc3, Wdst3, attn3)
    key = (N, IN, L["NCH"])
    if key not in _BUILD_CACHE:
        _BUILD_CACHE[key] = build_kernel(N, IN, L)
    nc = _BUILD_CACHE[key]
    res = run_bass_kernel_spmd(nc, in_maps, list(range(NCORE)), trace=trace,
                               trace_cores=list(range(NCORE)) if trace else None)
    out = np.concatenate([res.results[c]["out"] for c in range(NCORE)], axis=0)
    return out, res


def kernel(features, src, dst,
           Wsrc1, Wdst1, attn1, b1, Wres1,
           Wsrc2, Wdst2, attn2, b2,
           Wsrc3, Wdst3, attn3, b3):
    """Full-input entry point. Biases are zeros in this model (asserted)."""
    for b in (b1, b2, b3):
        assert float(np.abs(np.asarray(b)).max()) == 0.0, "nonzero bias unsupported"
    out, _ = run(np.asarray(features, np.float32), np.asarray(src), np.asarray(dst),
                 Wsrc1, Wdst1, attn1, Wres1, Wsrc2, Wdst2, attn2,
                 Wsrc3, Wdst3, attn3)
    return out.astype(np.float32)



# revision 1
# speedup vs baseline: 4.4594x; 4.4594x over previous
"""GATv2 (3-layer, 8-head) on 8 Trainium2 NeuronCores.

Strategy (edge-parallel, dst-sharded):
- Core c owns destination nodes [c*N/8, (c+1)*N/8) and all edges into them.
- Host sorts each core's edges by (dst-window, src-half), pads to 128-edge
  chunks with a chunk structure made uniform across cores (SPMD: one program).
- Per layer: fs = h@Wsrc for ALL nodes (layer 0: replicated GEMM from the
  replicated features; layers 1/2: sharded GEMM + AllGather), fd = h@Wdst for
  the local shard only.
- Edge phase per 128-dst window: dma_gather fs[src] rows (the only per-edge
  gather), expand fd[dst] via one-hot matmul, score
  s = attn . leaky_relu(fs+fd) via DVE mul+reduce, ex = exp(s) (no segment-max:
  scores are O(1) so exp is safe), unnormalized aggregation
  rstU = OneHot @ [ex*z | ex] via TensorE (denominator rides in the last 8
  columns), then per-window normalization rst = rstU/denom - fd (using
  sum_e ex*fd[dst] = denom*fd[v]) + residual, relu.
- Output: mean over heads, host concatenates the 8 dst shards.
"""
import sys
sys.path.insert(0, "/opt/trn_rl_repo")
import numpy as np
import concourse.bass as bass
import concourse.mybir as mybir
import concourse.tile as tile
from concourse import bacc
from concourse.bass_utils import run_bass_kernel_spmd

P = 128
NCORE = 8
SLOPE = 0.2
H = 8

F32 = mybir.dt.float32
BF16 = mybir.dt.bfloat16
I16 = mybir.dt.int16
AX = mybir.AxisListType
OP = mybir.AluOpType
AF = mybir.ActivationFunctionType


# ---------------------------------------------------------------- host layout
def build_layout(src, dst, N):
    """Edge layout: per-core, dst-window-sorted, src-half-split, 128-padded,
    chunk structure uniform across cores."""
    SH = N // NCORE
    NW = (SH + P - 1) // P
    HALF = min(32768, (N + 1) // 2)  # src-half split point (int16 idx range)
    cores = []
    for c in range(NCORE):
        m = (dst // SH) == c
        s, d = src[m], dst[m]
        dl = d - c * SH
        w = dl // P
        hf = (s >= HALF).astype(np.int64)
        order = np.lexsort((hf, w))
        s, dl, w, hf = s[order], dl[order], w[order], hf[order]
        groups = {}
        for wi in range(NW):
            for h in range(2):
                gm = (w == wi) & (hf == h)
                groups[(wi, h)] = (s[gm], dl[gm])
        cores.append(groups)
    # uniform chunk counts per (window, half)
    C = {}
    for wi in range(NW):
        for h in range(2):
            n = max(len(cores[c][(wi, h)][0]) for c in range(NCORE))
            C[(wi, h)] = (n + P - 1) // P
    NCH = sum(C.values())
    # flat edge arrays per core
    src_rel = np.zeros((NCORE, NCH * P), np.int16)
    dstw = np.full((NCORE, NCH * P), -1.0, np.float32)
    calls = []  # (half, chunk_off, n_chunks) shared structure
    off = 0
    for wi in range(NW):
        for h in range(2):
            nch = C[(wi, h)]
            g = 0
            while g < nch:
                gs = min(8, nch - g)
                calls.append((wi, h, off + g, gs))
                g += gs
            for c in range(NCORE):
                s, dl = cores[c][(wi, h)]
                k = off * P
                src_rel[c, k:k + len(s)] = (s - h * HALF).astype(np.int16)
                dstw[c, k:k + len(dl)] = (dl % P).astype(np.float32)
            off += nch
    assert off == NCH
    # wrapped int16 idx for dma_gather: per call, idx i -> [i%16, col+i//16]
    TOTC = NCH * P // 16
    idx_w = np.zeros((NCORE, P, TOTC), np.int16)
    for c in range(NCORE):
        w16 = src_rel[c].reshape(-1, 16).T  # [16, NCH*8]
        idx_w[c] = np.tile(w16, (8, 1))
    # chunk-column-major dstw: [P, NCH], edge k*P+p -> [p, k]
    dstw_cols = dstw.reshape(NCORE, NCH, P).transpose(0, 2, 1).copy()
    return dict(SH=SH, NW=NW, HALF=HALF, NCH=NCH, C=C, calls=calls,
                idx_w=idx_w, dstw_cols=dstw_cols)


# ---------------------------------------------------------------- bass kernel
def build_kernel(N, IN, L):
    """L = layout dict. IN = input feature dim (128)."""
    D = 256  # H*HID = H*OUT
    SH, NW, NCH = L["SH"], L["NW"], L["NCH"]
    HALF = L["HALF"]
    T0 = (N + P - 1) // P            # full-N tiles (layer-0 fs GEMM)
    SHP = NW * P                      # padded shard rows
    TOTC = NCH * P // 16

    nc = bacc.Bacc("TRN2", target_bir_lowering=False, debug=False,
                   num_devices=NCORE)
    # ---- inputs (per-core where noted)
    featT = nc.declare_dram_parameter("featT", [IN, N], F32, isOutput=False)
    featT_loc = nc.declare_dram_parameter("featT_loc", [IN, SH], F32, isOutput=False)
    Ws = [nc.declare_dram_parameter(f"Wsrc{l}", [IN if l == 0 else D, D], F32, isOutput=False) for l in range(3)]
    Wd = [nc.declare_dram_parameter(f"Wdst{l}", [IN if l == 0 else D, D], F32, isOutput=False) for l in range(3)]
    Wres1 = nc.declare_dram_parameter("Wres1", [IN, D], F32, isOutput=False)
    attn4 = [nc.declare_dram_parameter(f"attn4_{l}", [P, 4 * D], F32, isOutput=False) for l in range(3)]
    iota4 = nc.declare_dram_parameter("iota4", [P, 4 * P], F32, isOutput=False)
    ident_in = nc.declare_dram_parameter("ident", [P, P], F32, isOutput=False)
    idx_in = nc.declare_dram_parameter("idx_w", [P, TOTC], I16, isOutput=False)   # per-core
    dstw_in = nc.declare_dram_parameter("dstw", [P, NCH], F32, isOutput=False)    # per-core
    out_ext = nc.declare_dram_parameter("out", [SH, 32], F32, isOutput=True)

    with tile.TileContext(nc) as tc:
        with (
            tc.tile_pool(name="const", bufs=1) as cpool,
            tc.tile_pool(name="sbuf", bufs=4) as sb,
            tc.tile_pool(name="sb6", bufs=4) as sb6,
            tc.tile_pool(name="sb2", bufs=3) as sb2,
            tc.tile_pool(name="psum", bufs=2, space="PSUM") as ps,
            tc.tile_pool(name="dram", bufs=1, space="DRAM") as dr,
        ):
            # ---- persistent constants
            ident = cpool.tile([P, P], F32)
            nc.sync.dma_start(out=ident[:], in_=ident_in[:, :])
            ident16 = cpool.tile([P, P], BF16, tag="ident16")
            nc.vector.tensor_copy(out=ident16[:], in_=ident[:])
            iota_t = cpool.tile([P, 4 * P], F32)
            nc.sync.dma_start(out=iota_t[:], in_=iota4[:, :])
            idx_t = cpool.tile([P, TOTC], I16)
            nc.sync.dma_start(out=idx_t[:], in_=idx_in[:, :])
            dstw_t = cpool.tile([P, NCH], F32)
            nc.sync.dma_start(out=dstw_t[:], in_=dstw_in[:, :])
            attn_t = []
            for l in range(3):
                a32 = cpool.tile([P, 4 * D], F32, tag=f"attn32_{l}")
                nc.sync.dma_start(out=a32[:], in_=attn4[l][:, :])
                a = cpool.tile([P, 4 * D], BF16, tag=f"attn{l}")
                nc.vector.tensor_copy(out=a[:], in_=a32[:])
                attn_t.append(a)
            hT = cpool.tile([P, 2, SHP], F32, tag="hT")  # local shard, transposed

            # ---- DRAM internals
            fs_full0 = dr.tile([T0 * P, D], BF16, tag="fsf0")
            fs_full1 = dr.tile([N, D], BF16, tag="fsf1", addr_space="Shared")
            fs_full2 = dr.tile([N, D], BF16, tag="fsf2", addr_space="Shared")
            fs_full_l = [None, fs_full1, fs_full2]
            ag_in = dr.tile([SH, D], BF16, tag="agin")
            fd_dram = dr.tile([SHP, D], BF16, tag="fd")
            res_dram = dr.tile([SHP, D], F32, tag="res")
            h_dram = dr.tile([SHP, D], F32, tag="h")

            zero_sb = cpool.tile([P, D], F32, tag="zero")
            nc.vector.memset(zero_sb[:], 0.0)
            zero16 = cpool.tile([P, D], BF16, tag="zero16")
            nc.vector.memset(zero16[:], 0.0)
            if SHP > SH:  # zero the padded tails once
                nc.sync.dma_start(out=fd_dram[SH:SHP, :], in_=zero16[:SHP - SH, :])
                nc.sync.dma_start(out=res_dram[SH:SHP, :], in_=zero_sb[:SHP - SH, :])
                nc.sync.dma_start(out=h_dram[SH:SHP, :], in_=zero_sb[:SHP - SH, :])

            def gemm(lhsT_ap_fn, kchunks, rhs_t, mt, out_psum):
                """out_psum[mt, D] = sum_k lhsT_k.T @ rhs_k"""
                for k in range(kchunks):
                    nc.tensor.matmul(out_psum[:mt, :D], lhsT=lhsT_ap_fn(k),
                                     rhs=rhs_t[:, k, :],
                                     start=(k == 0), stop=(k == kchunks - 1))

            def load_w(wparam, kchunks, tag):
                wt = cpool.tile([P, kchunks, D], F32, tag=tag)
                nc.sync.dma_start(
                    out=wt[:], in_=wparam.ap().rearrange("(c k) n -> k c n", k=P))
                return wt

            wsrc_t = [load_w(Ws[l], 1 if l == 0 else 2, f"wsrc{l}") for l in range(3)]
            wdst_t = [load_w(Wd[l], 1 if l == 0 else 2, f"wdst{l}") for l in range(3)]
            wres_t = load_w(Wres1, 1, "wres")

            for l in range(3):
                DIN = IN if l == 0 else D
                KCH = DIN // P
                act_relu = l < 2
                # ================= dense phase =================
                if l == 0:
                    # replicated fs_full GEMM from featT
                    for t in range(T0):
                        mt = min(P, N - t * P)
                        lt = sb.tile([P, P], F32, tag="lhsT")
                        nc.sync.dma_start(out=lt[:, :mt], in_=featT[:, t * P:t * P + mt])
                        pst = ps.tile([P, 264], F32, tag="rstcat", space="PSUM")
                        nc.tensor.matmul(pst[:mt, :D], lhsT=lt[:, :mt],
                                         rhs=wsrc_t[0][:, 0, :], start=True, stop=True)
                        ot = sb.tile([P, D], BF16, tag="gout")
                        nc.scalar.copy(out=ot[:mt, :], in_=pst[:mt, :D])
                        nc.sync.dma_start(out=fs_full0[t * P:t * P + mt, :], in_=ot[:mt, :])
                    # sharded fd / res GEMMs from featT_loc
                    for w in range(NW):
                        mt = min(P, SH - w * P)
                        lt = sb.tile([P, P], F32, tag="lhsT")
                        nc.sync.dma_start(out=lt[:, :mt], in_=featT_loc[:, w * P:w * P + mt])
                        for (rhs_t, dest, dt_) in ((wdst_t[0], fd_dram, BF16), (wres_t, res_dram, F32)):
                            pst = ps.tile([P, 264], F32, tag="rstcat", space="PSUM")
                            nc.tensor.matmul(pst[:mt, :D], lhsT=lt[:, :mt],
                                             rhs=rhs_t[:, 0, :], start=True, stop=True)
                            ot = sb.tile([P, D], dt_, tag="gout" if dt_ is BF16 else "gout32")
                            nc.scalar.copy(out=ot[:mt, :], in_=pst[:mt, :D])
                            nc.sync.dma_start(out=dest[w * P:w * P + mt, :], in_=ot[:mt, :])
                else:
                    # sharded fs -> ag_in; fd -> fd_dram (lhsT = resident hT)
                    for w in range(NW):
                        mt = min(P, SH - w * P)
                        for (rhs_t, dest) in ((wsrc_t[l], ag_in), (wdst_t[l], fd_dram)):
                            pst = ps.tile([P, 264], F32, tag="rstcat", space="PSUM")
                            gemm(lambda k: hT[:, k, w * P:w * P + mt], KCH, rhs_t, mt, pst)
                            ot = sb.tile([P, D], BF16, tag="gout")
                            nc.scalar.copy(out=ot[:mt, :], in_=pst[:mt, :D])
                            nc.sync.dma_start(out=dest[w * P:w * P + mt, :], in_=ot[:mt, :])
                    nc.gpsimd.collective_compute(
                        "AllGather", OP.bypass,
                        replica_groups=[list(range(NCORE))],
                        ins=[ag_in.opt()], outs=[fs_full_l[l].opt()],
                    )
                table = fs_full0 if l == 0 else fs_full_l[l]
                res_src = res_dram if l == 0 else h_dram

                # ================= edge phase =================
                cur_w = -1
                rst_ps = None
                calls = L["calls"]
                for ci, (wi, hf, koff, gcs) in enumerate(calls):
                    if wi != cur_w:
                        cur_w = wi
                        rst_ps = ps.tile([P, 264], F32, tag="rstcat", space="PSUM")
                        fdw = sb2.tile([P, D], BF16, tag="fdw")
                        nc.sync.dma_start(out=fdw[:], in_=fd_dram[wi * P:wi * P + P, :])
                        first_mm = True
                    # gather fs rows for up to 8 chunks per call
                    z8 = sb6.tile([P, 8, D], BF16, tag="z")
                    tab = table[:, :] if hf == 0 else table[HALF:, :]
                    nc.gpsimd.dma_gather(
                        z8[:, :gcs, :], tab, idx_t[:, koff * 8:koff * 8 + gcs * 8],
                        gcs * P, gcs * P, D, single_packet=False)
                    last_call = ci + 1 == len(calls) or calls[ci + 1][0] != wi
                    for sub in range(0, gcs, 4):
                        gs = min(4, gcs - sub)
                        ko = koff + sub
                        z = z8[:, sub:sub + 4, :]
                        # one-hot (edges on partitions)
                        oh = sb.tile([P, 4, P], BF16, tag="oh")
                        nc.vector.tensor_tensor(
                            out=oh[:, :gs, :],
                            in0=dstw_t[:, ko:ko + gs].to_broadcast([P, gs, P]),
                            in1=iota_t[:].rearrange("p (g j) -> p g j", g=4)[:, :gs, :],
                            op=OP.is_equal)
                        # transposed one-hot (dst on partitions) via PE
                        ohT_ps = ps.tile([P, 4 * P], BF16, tag="ohT", space="PSUM")
                        for j in range(gs):
                            nc.tensor.transpose(out=ohT_ps[:, j * P:(j + 1) * P],
                                                in_=oh[:, j, :], identity=ident16[:])
                        ohT = sb.tile([P, 4 * P], BF16, tag="ohTs")
                        nc.scalar.copy(out=ohT[:, :gs * P], in_=ohT_ps[:, :gs * P])
                        # z_psum = OneHot_ve.T @ fdw + fs  (= fs[src]+fd[dst])
                        zps = ps.tile([P, 4, D], F32, tag="zps", space="PSUM")
                        for j in range(gs):
                            nc.tensor.matmul(zps[:, j, :], lhsT=ohT[:, j * P:(j + 1) * P],
                                             rhs=fdw[:], start=True, stop=False)
                            nc.tensor.matmul(zps[:, j, :], lhsT=ident16[:],
                                             rhs=z[:, j, :], start=False, stop=True)
                        # leaky-relu = max(z, 0.2z): ACT Copy(scale) + DVE max
                        # (keeps ACT on one LUT set: Copy/Relu/Exp -> no table reloads)
                        lrs = sb.tile([P, 4, D], BF16, tag="lrs")
                        nc.scalar.activation(lrs[:, :gs, :], zps[:, :gs, :], AF.Copy,
                                             scale=SLOPE)
                        lr = sb.tile([P, 4, D], BF16, tag="lr")
                        nc.vector.tensor_tensor(out=lr[:, :gs, :], in0=zps[:, :gs, :],
                                                in1=lrs[:, :gs, :], op=OP.max)
                        sm = sb.tile([P, 4, D], BF16, tag="sm")
                        nc.vector.tensor_tensor(
                            out=sm[:, :gs, :], in0=lr[:, :gs, :],
                            in1=attn_t[l][:].rearrange("p (g d) -> p g d", g=4)[:, :gs, :],
                            op=OP.mult)
                        sc = sb.tile([P, 4, H], F32, tag="sc")
                        nc.vector.tensor_reduce(
                            out=sc[:, :gs, :],
                            in_=sm[:, :gs, :].rearrange("p g (h d) -> p g h d", h=H),
                            axis=AX.X, op=OP.add)
                        wcat = sb.tile([P, 4, 264], BF16, tag="wcat")
                        nc.scalar.activation(wcat[:, :gs, D:D + H], sc[:, :gs, :], AF.Exp)
                        # W = ex * z
                        nc.vector.tensor_tensor(
                            out=wcat[:, :gs, :D].rearrange("p g (h d) -> p g h d", h=H),
                            in0=zps[:, :gs, :].rearrange("p g (h d) -> p g h d", h=H),
                            in1=wcat[:, :gs, D:D + H].to_broadcast([P, gs, H, 32]),
                            op=OP.mult)
                        # accumulate [rstU | denom]
                        for j in range(gs):
                            last = last_call and sub + gs >= gcs and j == gs - 1
                            nc.tensor.matmul(rst_ps[:, :], lhsT=oh[:, j, :],
                                             rhs=wcat[:, j, :], start=first_mm, stop=last)
                            first_mm = False
                    # window epilogue
                    if ci + 1 == len(calls) or calls[ci + 1][0] != wi:
                        wt = min(P, SH - wi * P)
                        den = sb2.tile([P, H], F32, tag="den")
                        nc.vector.tensor_scalar_max(den[:], rst_ps[:, D:D + H], 1e-30)
                        rec = sb2.tile([P, H], F32, tag="rec")
                        nc.vector.reciprocal(rec[:], den[:])
                        msk = sb2.tile([P, H], F32, tag="msk")
                        nc.vector.tensor_scalar(out=msk[:], in0=rst_ps[:, D:D + H],
                                                scalar1=1e30, scalar2=1.0,
                                                op0=OP.mult, op1=OP.min)
                        rn = sb2.tile([P, D], F32, tag="rn")
                        nc.vector.tensor_tensor(
                            out=rn[:].rearrange("p (h d) -> p h d", h=H),
                            in0=rst_ps[:, :D].rearrange("p (h d) -> p h d", h=H),
                            in1=rec[:].to_broadcast([P, H, 32]), op=OP.mult)
                        fdw32 = sb2.tile([P, D], F32, tag="fdw32")
                        nc.vector.tensor_copy(out=fdw32[:], in_=fdw[:])
                        fdm = sb2.tile([P, D], F32, tag="fdm")
                        nc.vector.tensor_tensor(
                            out=fdm[:].rearrange("p (h d) -> p h d", h=H),
                            in0=fdw32[:].rearrange("p (h d) -> p h d", h=H),
                            in1=msk[:].to_broadcast([P, H, 32]), op=OP.mult)
                        nc.vector.tensor_tensor(out=rn[:], in0=rn[:], in1=fdm[:],
                                                op=OP.subtract)
                        rt = sb2.tile([P, D], F32, tag="rt")
                        nc.sync.dma_start(out=rt[:], in_=res_src[wi * P:wi * P + P, :])
                        nc.vector.tensor_tensor(out=rn[:], in0=rn[:], in1=rt[:], op=OP.add)
                        hsb = sb2.tile([P, D], F32, tag="hsb")
                        if act_relu:
                            nc.scalar.activation(hsb[:], rn[:], AF.Relu)
                        else:
                            nc.vector.tensor_copy(out=hsb[:], in_=rn[:])
                        if l < 2:
                            nc.sync.dma_start(out=h_dram[wi * P:wi * P + wt, :],
                                              in_=hsb[:wt, :])
                            for half in range(2):
                                tp = ps.tile([P, 4 * P], F32, tag="ohT", space="PSUM")
                                nc.tensor.transpose(out=tp[:, :P],
                                                    in_=hsb[:, half * P:(half + 1) * P],
                                                    identity=ident[:])
                                nc.scalar.copy(out=hT[:, half, wi * P:(wi + 1) * P],
                                               in_=tp[:, :P])
                        else:
                            mean = sb2.tile([P, 32], F32, tag="mean")
                            nc.vector.tensor_reduce(
                                out=mean[:],
                                in_=hsb[:].rearrange("p (h d) -> p d h", h=H),
                                axis=AX.X, op=OP.add)
                            osb = sb2.tile([P, 32], F32, tag="osb")
                            nc.scalar.mul(osb[:], mean[:], 1.0 / H)
                            nc.sync.dma_start(out=out_ext[wi * P:wi * P + wt, :],
                                              in_=osb[:wt, :])
    nc.compile()
    return nc


# ---------------------------------------------------------------- host driver
def prep_inputs(features, src, dst, Wsrc1, Wdst1, attn1, Wres1,
                Wsrc2, Wdst2, attn2, Wsrc3, Wdst3, attn3):
    N, IN = features.shape
    L = build_layout(np.asarray(src), np.asarray(dst), N)
    featT = np.ascontiguousarray(np.asarray(features).T)
    SH = L["SH"]

    def attn_rep(a):
        flat = np.asarray(a).reshape(-1)  # [256]
        return np.tile(np.tile(flat, 4)[None, :], (P, 1)).astype(np.float32)

    iota = np.tile(np.arange(P, dtype=np.float32)[None, :], (P, 4))
    ident = np.eye(P, dtype=np.float32)
    common = {
        "featT": featT, "ident": ident, "iota4": iota,
        "Wsrc0": np.asarray(Wsrc1), "Wdst0": np.asarray(Wdst1), "Wres1": np.asarray(Wres1),
        "Wsrc1": np.asarray(Wsrc2), "Wdst1": np.asarray(Wdst2),
        "Wsrc2": np.asarray(Wsrc3), "Wdst2": np.asarray(Wdst3),
        "attn4_0": attn_rep(attn1), "attn4_1": attn_rep(attn2), "attn4_2": attn_rep(attn3),
    }
    in_maps = []
    for c in range(NCORE):
        m = dict(common)
        m["featT_loc"] = np.ascontiguousarray(featT[:, c * SH:(c + 1) * SH])
        m["idx_w"] = L["idx_w"][c]
        m["dstw"] = L["dstw_cols"][c]
        in_maps.append(m)
    return L, in_maps


_BUILD_CACHE = {}


def run(features, src, dst, Wsrc1, Wdst1, attn1, Wres1,
        Wsrc2, Wdst2, attn2, Wsrc3, Wdst3, attn3, trace=False):
    N, IN = features.shape
    L, in_maps = prep_inputs(features, src, dst, Wsrc1, Wdst1, attn1, Wres1,
                             Wsrc2, Wdst2, attn2, Wsrc3, Wdst3, attn3)
    key = (N, IN, L["NCH"])
    if key not in _BUILD_CACHE:
        _BUILD_CACHE[key] = build_kernel(N, IN, L)
    nc = _BUILD_CACHE[key]
    res = run_bass_kernel_spmd(nc, in_maps, list(range(NCORE)), trace=trace,
                               trace_cores=list(range(NCORE)) if trace else None)
    out = np.concatenate([res.results[c]["out"] for c in range(NCORE)], axis=0)
    return out, res


def kernel(features, src, dst,
           Wsrc1, Wdst1, attn1, b1, Wres1,
           Wsrc2, Wdst2, attn2, b2,
           Wsrc3, Wdst3, attn3, b3):
    """Full-input entry point. Biases are zeros in this model (asserted)."""
    for b in (b1, b2, b3):
        assert float(np.abs(np.asarray(b)).max()) == 0.0, "nonzero bias unsupported"
    out, _ = run(np.asarray(features, np.float32), np.asarray(src), np.asarray(dst),
                 Wsrc1, Wdst1, attn1, Wres1, Wsrc2, Wdst2, attn2,
                 Wsrc3, Wdst3, attn3)
    return out.astype(np.float32)

